# revision 12
# baseline (speedup 1.0000x reference)
"""EuclideanCodebook (VQ) kernel for 8 TRN2 NeuronCores.

Problem: x [8, 4096, 256] f32, embedding [2048, 256] f32.
  xf = x.reshape(-1, 256)                       # [32768, 256]
  dist = |xf|^2 + |e|^2 - 2 xf @ e.T            # [32768, 2048]
  codes = argmin(dist, -1)                      # [32768] int32
  quantized = embedding[codes]                  # [32768, 256]
  returns (quantized, xf, codes)

Sharding: data-parallel on tokens, 4096 per core; the embedding is
replicated. The host supplies each shard both in natural and in
transposed layout (pure data movement) so the device needs no on-chip
transposes; all arithmetic of the reference (x.e matmul, |e|^2, the
bias add, argmin, gather) runs on the device:

  stage A (once): DMA e.T chunk tiles [128d x 512k]; ACT Square+accum
  over e tiles -> |e_k|^2; PE transpose + matmul-broadcast -> nhb2
  chunk tiles holding -|e|^2/2 on all 128 partitions.

  main loop (32 token tiles of 128 tokens):
   - DMA xT tile [128d x 128tok x 2 chunks] from the transposed shard
   - per 512-code chunk: ACT preloads PSUM with -|e|^2/2, then 2 fp32
     matmuls accumulate x.e on top (start=False); ACT copies the
     finished scores chunk to SBUF.  argmax(score) == argmin(dist).
   - DVE max8 + find_index8 over scores [128, 2048] -> code per token
   - gpsimd indirect DMA gathers embedding[code] -> quantized rows
   - q rows + codes column DMA out (sync queue)

Token <-> (partition, tile) mapping: local token n = t*128 + p, so
every large DMA (xT slices, q rows) is partition-contiguous.
"""

import numpy as np
from contextlib import ExitStack

P = 128
D = 256
K = 2048
N_CORES = 8
TOK_PER_CORE = 4096
NCHUNK = 4  # 2048 / 512 free-dim chunks
CW = 512
KCH = 2  # 256 / 128 contraction chunks

MM_DTYPE = "float32"


def build(n_tok=TOK_PER_CORE, mm_dtype=MM_DTYPE):
    import concourse.bass as bass
    import concourse.tile as tile
    from concourse import bacc, mybir
    from concourse.bass import ts
    from concourse.masks import make_identity

    f32 = mybir.dt.float32
    u32 = mybir.dt.uint32
    mmdt = getattr(mybir.dt, mm_dtype)

    TT = n_tok // P  # token tiles

    nc = bacc.Bacc(
        "TRN2", target_bir_lowering=False, debug=False, num_devices=N_CORES
    )

    xt_d = nc.dram_tensor("x_t", [D, n_tok], f32, kind="ExternalInput").ap()
    e_d = nc.dram_tensor("emb", [K, D], f32, kind="ExternalInput").ap()
    et_d = nc.dram_tensor("emb_t", [D, K], f32, kind="ExternalInput").ap()
    q_d = nc.dram_tensor("q", [n_tok, D], f32, kind="ExternalOutput").ap()
    c_d = nc.dram_tensor("codes", [TT, P], u32, kind="ExternalOutput").ap()

    with tile.TileContext(nc) as tc, ExitStack() as ctx:
        const = ctx.enter_context(tc.tile_pool(name="const", bufs=1))
        epool = ctx.enter_context(tc.tile_pool(name="epool", bufs=3))
        xTpool = ctx.enter_context(tc.tile_pool(name="xTpool", bufs=3))
        spool = ctx.enter_context(tc.tile_pool(name="spool", bufs=3))
        qpool = ctx.enter_context(tc.tile_pool(name="qpool", bufs=3))
        ipool = ctx.enter_context(tc.tile_pool(name="ipool", bufs=4))
        junk = ctx.enter_context(tc.tile_pool(name="junk", bufs=2))
        psum_b2 = ctx.enter_context(tc.tile_pool(name="psum_b2", bufs=2, space="PSUM"))
        psum_mm = ctx.enter_context(tc.tile_pool(name="psum_mm", bufs=6, space="PSUM"))

        identity = const.tile([P, P], f32, tag="identity")
        make_identity(nc, identity[:])
        ones_row = const.tile([1, P], f32, tag="ones_row")
        nc.gpsimd.memset(ones_row[:], 1.0)
        eT = [
            [
                const.tile([P, CW], f32, tag=f"eT{c}_{n}", name=f"eT{c}_{n}")
                for n in range(NCHUNK)
            ]
            for c in range(KCH)
        ]
        nhb2 = [
            const.tile([P, CW], f32, tag=f"nhb2_{n}", name=f"nhb2_{n}")
            for n in range(NCHUNK)
        ]
        b2c = const.tile([P, K // P], f32, tag="b2c")
        nhb2_row = const.tile([1, K], f32, tag="nhb2_row")

        # ---- stage A ----
        for c in range(KCH):
            for nch in range(NCHUNK):
                nc.sync.dma_start(
                    eT[c][nch][:], et_d[ts(c, P), ts(nch, CW)]
                )
        for kt in range(K // P):
            et = epool.tile([P, D], f32, tag="et")
            nc.sync.dma_start(et[:], e_d[ts(kt, P), :])
            sqj = junk.tile([P, D], f32, tag="sqj")
            nc.scalar.activation(
                sqj[:],
                et[:],
                mybir.ActivationFunctionType.Square,
                accum_out=b2c[:, kt : kt + 1],
            )
        psb = psum_b2.tile([K // P, P], f32, tag="b2t")
        nc.tensor.transpose(psb[:], b2c[:], identity[:])
        b2T = junk.tile([K // P, P], f32, tag="b2T")
        nc.scalar.mul(b2T[:], psb[:], -0.5)
        nc.sync.dma_start(
            nhb2_row[0:1, :].rearrange("o (a b) -> o a b", a=K // P), b2T[:]
        )
        for nch in range(NCHUNK):
            bb = psum_mm.tile([P, CW], f32, tag="mm")
            nc.tensor.matmul(
                bb[:],
                lhsT=ones_row[:],
                rhs=nhb2_row[0:1, ts(nch, CW)],
                start=True,
                stop=True,
            )
            nc.scalar.copy(nhb2[nch][:], bb[:])

        # ---- main loop over token tiles ----
        for t in range(TT):
            xT = xTpool.tile([P, D], f32, tag="xT")
            for c in range(KCH):
                nc.sync.dma_start(xT[:, ts(c, P)], xt_d[ts(c, P), ts(t, P)])

            scores = spool.tile([P, K], f32, tag="scores")
            for nch in range(NCHUNK):
                mm = psum_mm.tile([P, CW], f32, tag="mm")
                nc.scalar.copy(mm[:], nhb2[nch][:])  # PSUM = -|e|^2/2
                for c in range(KCH):
                    nc.tensor.matmul(
                        mm[:],
                        lhsT=xT[:, ts(c, P)].bitcast(mmdt),
                        rhs=eT[c][nch][:].bitcast(mmdt),
                        start=False,
                        stop=(c == KCH - 1),
                        skip_group_check=True,
                    )
                nc.scalar.copy(scores[:, ts(nch, CW)], mm[:])

            v8 = ipool.tile([P, 8], f32, tag="v8")
            nc.vector.max(v8[:], scores[:])
            idx8 = ipool.tile([P, 8], u32, tag="idx8")
            nc.vector.max_index(idx8[:], v8[:], scores[:])

            qt = qpool.tile([P, D], f32, tag="qt")
            nc.gpsimd.indirect_dma_start(
                out=qt[:],
                out_offset=None,
                in_=e_d[:, :],
                in_offset=bass.IndirectOffsetOnAxis(ap=idx8[:, 0:1], axis=0),
            )
            nc.sync.dma_start(q_d[ts(t, P), :], qt[:])
            nc.sync.dma_start(c_d[t : t + 1, :], idx8[:, 0:1])

    nc.compile()
    return nc


def kernel(x, embedding):
    from concourse.bass_utils import run_bass_kernel_spmd

    x = np.ascontiguousarray(np.asarray(x, dtype=np.float32))
    e = np.ascontiguousarray(np.asarray(embedding, dtype=np.float32))
    xf = x.reshape(-1, D)
    n_total = xf.shape[0]
    assert n_total == N_CORES * TOK_PER_CORE and e.shape == (K, D)

    e_t = np.ascontiguousarray(e.T)
    nc = build()
    in_maps = []
    for i in range(N_CORES):
        shard = xf[i * TOK_PER_CORE : (i + 1) * TOK_PER_CORE]
        in_maps.append(
            {"x_t": np.ascontiguousarray(shard.T), "emb": e, "emb_t": e_t}
        )
    res = run_bass_kernel_spmd(nc, in_maps, list(range(N_CORES))).results

    q = np.concatenate([res[i]["q"] for i in range(N_CORES)], axis=0)
    codes = np.concatenate(
        [res[i]["codes"].reshape(-1) for i in range(N_CORES)], axis=0
    ).astype(np.int32)
    return q, xf, codes
